# revision 44
# baseline (speedup 1.0000x reference)
"""Distributed Trainium2 kernel for the AttentionBlock problem.

Full inputs:
  x     [4, 2048, 512] f32
  w_qkv [512, 1536]    f32   (columns: q | k | v, each 512 wide)
  w_out [512, 512]     f32
  b_out [512]          f32

Sharding over 8 cores: core c handles batch (c // 2) and head-group
(c % 2) -> 4 heads of 64 dims each (feature slice of 256 per section).
Each core computes a partial output projection (its 4 heads' contribution
to out = attn @ w_out); the host sums the two partials per batch and adds
the bias.

Per-core dataflow (bf16 matmuls, all intermediates in SBUF):
  xT [512, 2048] -> qT,kT [256+256, 2048] (transposed layout, bf16)
                 -> v natural [2048, 4*65] (65th col per head = ones)
  attention, processed per (query-half of 1024) x (head pair):
    S^T[j-block 128, i 512x2] = kT.T @ qT  two heads packed side by side
                                           (64x128 PE row tiles, concurrent)
    P^T = exp(S^T * 0.125)                 softmax exp split between
                                           ScalarE (exact LUT) and VectorE
                                           (Schraudolph int16 bit-trick,
                                           ~1.8% rms, on a minority of
                                           chunks) to balance engine load
    outT[f 65, i 512] += v'.T @ P^T        (v' stationary; j-major PSUM
                                            accumulation; row 64 = denom l)
    attnT[f, i] = outT[0:64] * (1/l)       (DVE recip straight from PSUM +
                                            gpsimd broadcast + DVE mul)
  out_partial[i 128, 512] = attnT.T @ w_out -> DRAM, with the final
  projection MMs interleaved into the next query-half's j-loop so the
  PE never sits idle at phase boundaries.
"""

import sys

if "/opt/trn_rl_repo" not in sys.path:
    sys.path.insert(0, "/opt/trn_rl_repo")

import numpy as np

DIM = 512
HEADS = 8
DIM_HEAD = 64
INNER = 512
B, N = 4, 2048
N_CORES = 8
HEADS_PER_CORE = 4
FEAT = HEADS_PER_CORE * DIM_HEAD  # 256 features per core per section
SCALE = DIM_HEAD ** -0.5  # 0.125

N_JB = N // 128  # 16 j-blocks

# Schraudolph approximate-exp constants (bf16 bit-trick):
#   bits16(y) = round(A_S * s + B_S),  y ~= exp(SCALE * s)
# A = SCALE * 2^7 * log2(e); B = 127*2^7 - C with C tuned for min rms
# relative error (~1.8 % rms, 4.2 % max).
A_SCHR = SCALE * 128.0 * 1.4426950408889634
B_SCHR = 16256.0 - 7.5

# Every exp chunk is split column-wise across both elementwise engines:
# ScalarE computes the first EXP_SPLIT columns with its exact LUT while
# VectorE computes the rest via Schraudolph, so the per-chunk exp latency
# is roughly halved and neither engine paces the pipeline alone.
EXP_SPLIT = 576

_CACHED = {}


def _build():
    import concourse.mybir as mybir
    import concourse.tile as tile
    from concourse import bacc

    f32 = mybir.dt.float32
    bf16 = mybir.dt.bfloat16
    i16 = mybir.dt.int16
    EXP = mybir.ActivationFunctionType.Exp
    MUL = mybir.AluOpType.mult
    ADD = mybir.AluOpType.add

    nc = bacc.Bacc("TRN2", target_bir_lowering=False, debug=False,
                   num_devices=N_CORES)

    xT = nc.declare_dram_parameter("xT", [DIM, N], f32, isOutput=False)
    wqk = nc.declare_dram_parameter("wqk", [DIM, 2 * FEAT], f32, isOutput=False)
    wv = nc.declare_dram_parameter("wv", [DIM, FEAT], f32, isOutput=False)
    w_out = nc.declare_dram_parameter("w_out", [FEAT, DIM], f32, isOutput=False)
    out = nc.declare_dram_parameter("out", [N, DIM], f32, isOutput=True)

    with tile.TileContext(nc) as tc:
        with (
            tc.tile_pool(name="xb", bufs=1) as xb_pool,
            tc.tile_pool(name="raw", bufs=6) as raw_pool,
            tc.tile_pool(name="wraw", bufs=2) as wraw_pool,
            tc.tile_pool(name="wq", bufs=1) as w_pool,
            tc.tile_pool(name="qkt", bufs=1) as qkt_pool,
            tc.tile_pool(name="vs", bufs=1) as v_pool,
            tc.tile_pool(name="pt", bufs=8) as pt_pool,
            tc.tile_pool(name="attnT", bufs=1) as attnT_pool,
            tc.tile_pool(name="scl", bufs=8) as scl_pool,
            tc.tile_pool(name="dout", bufs=3) as dout_pool,
            tc.tile_pool(name="mm", bufs=3, space="PSUM") as mm_psum,
            tc.tile_pool(name="pv", bufs=2, space="PSUM") as pv_psum,
        ):
            # preload the exp table-set while DMAs run so the first real
            # softmax chunk doesn't pay the ~2.7us ACT_TABLE_LOAD stall
            warm = w_pool.tile([1, 16], f32, tag="warm", name="warm")
            nc.vector.memset(warm[:], 0.0)
            nc.scalar.activation(warm[:], warm[:], EXP)

            # ---- weights: load + convert to bf16 ----
            wqk_t = []
            for k in range(4):
                raw = wraw_pool.tile([128, 2 * FEAT], f32, tag="wraw", name=f"qkraw{k}")
                nc.sync.dma_start(out=raw[:], in_=wqk[k * 128:(k + 1) * 128, :])
                t = w_pool.tile([128, 2 * FEAT], bf16, tag=f"wqk{k}", name=f"wqkb{k}")
                nc.vector.tensor_copy(t[:], raw[:])
                wqk_t.append(t)
            # wv / w_out loads are deferred until after the first x block's
            # DMA so the q/k projection starts as early as possible
            wv_t = []

            def load_wv():
                for k in range(4):
                    raw = wraw_pool.tile([128, FEAT], f32, tag="wraw",
                                         name=f"vraw{k}")
                    nc.sync.dma_start(out=raw[:],
                                      in_=wv[k * 128:(k + 1) * 128, :])
                    t = w_pool.tile([128, FEAT], bf16, tag=f"wv{k}",
                                    name=f"wvb{k}")
                    nc.vector.tensor_copy(t[:], raw[:])
                    wv_t.append(t)

            wo = []

            def load_wo():
                for k in range(2):
                    raw = wraw_pool.tile([128, DIM], f32, tag="wraw",
                                         name=f"woraw{k}")
                    nc.sync.dma_start(out=raw[:],
                                      in_=w_out[k * 128:(k + 1) * 128, :])
                    t = w_pool.tile([128, DIM], bf16, tag=f"wob{k}",
                                    name=f"wob{k}")
                    nc.vector.tensor_copy(t[:], raw[:])
                    wo.append(t)

            # ---- x load + projections, streamed by 512-column blocks ----
            # qkt[m] partition p = feature m*128+p of [q(256) | k(256)], bf16
            qkt = [qkt_pool.tile([128, N], bf16, tag=f"qkt{m}", name=f"qkt{m}")
                   for m in range(4)]
            # v natural layout with a ones column per head: [j 128, 4*65] bf16
            vt = [v_pool.tile([128, 4 * 65], bf16, tag=f"v{j}", name=f"v{j}")
                  for j in range(N_JB)]
            xb = [xb_pool.tile([128, N], bf16, tag=f"xb{k}", name=f"xb{k}")
                  for k in range(4)]

            def proj_qk(m, n, evac_dve=False):
                ncol = slice(n * 512, (n + 1) * 512)
                ps = mm_psum.tile([128, 512], f32, tag="qs", name="psb")
                for k in range(4):
                    nc.tensor.matmul(
                        ps[:],
                        wqk_t[k][:, m * 128:(m + 1) * 128],
                        xb[k][:, ncol],
                        start=(k == 0), stop=(k == 3),
                    )
                # pre-phase projections evacuate on the then-idle ScalarE;
                # the pops interleaved into the attention stream use DVE
                # (ScalarE is the busier engine there)
                if evac_dve:
                    nc.vector.tensor_copy(qkt[m][:, ncol], ps[:])
                else:
                    nc.scalar.copy(qkt[m][:, ncol], ps[:])

            def proj_v(j):
                ps = mm_psum.tile([128, 256], f32, tag="qs", name="psv")
                for k in range(4):
                    nc.tensor.matmul(
                        ps[:],
                        xb[k][:, j * 128:(j + 1) * 128],
                        wv_t[k][:],
                        start=(k == 0), stop=(k == 3),
                    )
                v_view = vt[j].rearrange("p (h f) -> p h f", h=4)
                nc.scalar.copy(
                    v_view[:, :, 0:64], ps.rearrange("p (h f) -> p h f", h=4)
                )
                nc.vector.memset(v_view[:, :, 64:65], 1.0)

            # pair-0 q/k and ALL of v first: everything attention pass 1
            # needs. pair-1's projections are deferred and interleaved into
            # the first attention pass via pending_proj.
            for n in range(4):
                ncol = slice(n * 512, (n + 1) * 512)
                for k in range(4):
                    raw = raw_pool.tile([128, 512], f32, tag="raw",
                                        name=f"xraw{n}{k}")
                    nc.sync.dma_start(out=raw[:],
                                      in_=xT[k * 128:(k + 1) * 128, ncol])
                    nc.vector.tensor_copy(xb[k][:, ncol], raw[:])
                if n == 0:
                    load_wv()
                proj_qk(0, n)
                proj_qk(2, n)
                for jj in range(4):
                    proj_v(4 * n + jj)
                if n == 0:
                    load_wo()
            pending_proj = [(1, 0), (3, 0), (1, 1), (3, 1),
                            (1, 2), (3, 2), (1, 3), (3, 3)]

            # ---- attention + output projection ----
            # attnT[t] partition p = output feature t*128+p, columns = queries
            attnT = [attnT_pool.tile([128, N], bf16, tag=f"aT{t}", name=f"aT{t}")
                     for t in range(2)]

            # Normalize in two stages. Evac: two base-aligned copies move the
            # accumulator out of PSUM right at the pass boundary (the big
            # attn-rows copy on DVE, whose queue is empty there; the denom
            # row on ScalarE) so the accumulator buffers recycle within ~1
            # chunk. Compute: reciprocal (custom DVE op, base-0 SBUF only),
            # gpsimd broadcast, and the multiply run later from SBUF, spread
            # into steps where the engines have slack.
            def norm_evac(ic_ps):
                lrow = scl_pool.tile([1, 512], f32, tag="lrow", name="lrow")
                nc.scalar.copy(lrow[:], ic_ps[64:65, :])
                a_s = scl_pool.tile([64, 512], f32, tag="as", name="a_s")
                nc.vector.tensor_copy(a_s[:], ic_ps[0:64, :])
                return lrow, a_s

            def norm_compute(pair, hh, lrow, a_s, i0):
                rl = scl_pool.tile([1, 512], f32, tag="rl", name="rl")
                nc.vector.reciprocal_approx_fast(rl[:], lrow[:])
                rlb = scl_pool.tile([64, 512], f32, tag="rlb", name="rlb")
                nc.gpsimd.partition_broadcast(rlb[:], rl[:])
                nc.vector.tensor_tensor(
                    attnT[pair][hh * 64:(hh + 1) * 64, i0:i0 + 512],
                    a_s[:], rlb[:], MUL,
                )

            def out_proj_block(ic):
                # one 128-query block of the final projection
                ps = mm_psum.tile([128, 512], f32, tag="qs", name="psd")
                for t in range(2):
                    nc.tensor.matmul(
                        ps[:],
                        attnT[t][:, ic * 128:(ic + 1) * 128],
                        wo[t][:],
                        start=(t == 0), stop=(t == 1),
                    )
                ot = dout_pool.tile([128, 512], f32, tag="ot", name="ot")
                # alternate the evacuation between the two elementwise
                # engines to keep their loads level
                if ic % 2 == 0:
                    nc.scalar.copy(ot[:], ps[:])
                else:
                    nc.vector.tensor_copy(ot[:], ps[:])
                nc.sync.dma_start(out=out[ic * 128:(ic + 1) * 128, :],
                                  in_=ot[:])

            # One continuous software-pipelined stream over all 8 passes x 16
            # key blocks = 128 chunks. Stages: S(c) | exp(c-1) | PV(c-2, one
            # extra step for DVE chunks). Pass boundaries are just parameter
            # changes - the PE queue never drains between passes. Deferred
            # queues keep the DVE/ACT streams dense:
            #   pending_out  - final-projection blocks, queued once the
            #                  normalize writing their attnT columns emitted
            pending_out = []

            passes = [(half, ic, pair)
                      for half in range(2) for ic in range(2)
                      for pair in range(2)]
            NP = len(passes)
            outp_of = {}
            pv_wait = {}
            pv_left = {p: N_JB for p in range(NP)}
            pv_first = {p: True for p in range(NP)}

            def emit_pv(c):
                qs_c, p_c, pi, j = pv_wait.pop(c)
                _, _, pair = passes[pi]
                outp = outp_of[pi]
                first = pv_first[pi]
                pv_first[pi] = False
                pv_left[pi] -= 1
                for hh in range(2):
                    h = 2 * pair + hh
                    nc.tensor.matmul(
                        outp[hh][0:65, :],
                        vt[j][:, h * 65:(h + 1) * 65],
                        p_c[:, hh * 512:(hh + 1) * 512],
                        start=first,
                        stop=(pv_left[pi] == 0),
                    )

            norm_compute_pending = []

            def queue_norms(pi):
                half, ic, pair = passes[pi]
                i0 = half * 1024 + ic * 512
                for hh in range(2):
                    lrow, a_s = norm_evac(outp_of[pi][hh])

                    def run(hh=hh, lrow=lrow, a_s=a_s, pair=pair, i0=i0,
                            half=half, ic=ic):
                        norm_compute(pair, hh, lrow, a_s, i0)
                        # the final projection for these attnT columns may
                        # only be EMITTED after the normalize writing them
                        # (Tile tracks deps by emission order).
                        if hh == 1 and pair == 1:
                            pending_out.extend(
                                range(half * 8 + ic * 4,
                                      half * 8 + ic * 4 + 4))
                    norm_compute_pending.append(run)

            NC_TOT = NP * N_JB
            for c in range(NC_TOT + 4):
                if c < NC_TOT:
                    pi, j = c // N_JB, c % N_JB
                    half, ic, pair = passes[pi]
                    if j == 0:
                        outp_of[pi] = [
                            pv_psum.tile([65, 512], f32, tag="pv",
                                         name=f"o{pi}{hh}")
                            for hh in range(2)]
                    qt, kt = qkt[pair], qkt[2 + pair]
                    i0 = half * 1024 + ic * 512
                    qs = mm_psum.tile([128, 1024], f32, tag="qs", name="qs")
                    # two heads concurrently in 64x128 PE row-tiles
                    for hh in range(2):
                        nc.tensor.matmul(
                            qs[:, hh * 512:(hh + 1) * 512],
                            kt[hh * 64:(hh + 1) * 64, j * 128:(j + 1) * 128],
                            qt[hh * 64:(hh + 1) * 64, i0:i0 + 512],
                            start=True, stop=True,
                        )
                    pv_wait[c] = [qs, None, pi, j]
                e = c - 1
                if 0 <= e < NC_TOT:
                    qs_e = pv_wait[e][0]
                    p = pt_pool.tile([128, 1024], bf16, tag="pt",
                                     name="ptile")
                    nc.scalar.activation(p[:, 0:EXP_SPLIT],
                                         qs_e[:, 0:EXP_SPLIT], EXP,
                                         scale=SCALE)
                    nc.vector.tensor_scalar(
                        p[:, EXP_SPLIT:1024].bitcast(i16),
                        qs_e[:, EXP_SPLIT:1024],
                        A_SCHR, B_SCHR, MUL, ADD,
                    )
                    pv_wait[e][1] = p
                for cc in sorted(pv_wait):
                    if cc <= c - 2:
                        emit_pv(cc)
                # a pass's accumulation just finished -> evacuate its
                # accumulators right away (frees the PSUM buffers for the
                # next pass within ~1 chunk); the normalize arithmetic is
                # deferred into later steps.
                if c >= 2 and (c - 2) % N_JB == N_JB - 1:
                    done_pi = (c - 2) // N_JB
                    if pv_left.get(done_pi) == 0:
                        queue_norms(done_pi)
                if c % 2 == 0 and c >= 2:
                    if norm_compute_pending and c % N_JB >= 4:
                        norm_compute_pending.pop(0)()
                    elif pending_proj:
                        proj_qk(*pending_proj.pop(0))
                    elif pending_out:
                        out_proj_block(pending_out.pop(0))
            while norm_compute_pending:
                norm_compute_pending.pop(0)()
            while pending_out:
                out_proj_block(pending_out.pop(0))

    nc.compile()
    return nc


def _get_nc():
    if "nc" not in _CACHED:
        _CACHED["nc"] = _build()
    return _CACHED["nc"]


def kernel(x, w_qkv, w_out, b_out):
    from concourse.bass_utils import run_bass_kernel_spmd

    x = np.asarray(x, dtype=np.float32)
    w_qkv = np.asarray(w_qkv, dtype=np.float32)
    w_out = np.asarray(w_out, dtype=np.float32)
    b_out = np.asarray(b_out, dtype=np.float32)

    in_maps = []
    for c in range(N_CORES):
        bi, hg = c // 2, c % 2
        f0 = hg * FEAT
        wq = w_qkv[:, f0:f0 + FEAT]
        wk = w_qkv[:, INNER + f0:INNER + f0 + FEAT]
        wvs = w_qkv[:, 2 * INNER + f0:2 * INNER + f0 + FEAT]
        in_maps.append({
            "xT": np.ascontiguousarray(x[bi].T),
            "wqk": np.ascontiguousarray(np.concatenate([wq, wk], axis=1)),
            "wv": np.ascontiguousarray(wvs),
            "w_out": np.ascontiguousarray(w_out[f0:f0 + FEAT, :]),
        })

    nc = _get_nc()
    res = run_bass_kernel_spmd(nc, in_maps, list(range(N_CORES)))

    outa = np.empty((B, N, DIM), dtype=np.float32)
    for bi in range(B):
        outa[bi] = (res.results[2 * bi]["out"]
                    + res.results[2 * bi + 1]["out"] + b_out)
    return outa


# revision 48
# speedup vs baseline: 1.0346x; 1.0346x over previous
"""Distributed Trainium2 kernel for the AttentionBlock problem.

Full inputs:
  x     [4, 2048, 512] f32
  w_qkv [512, 1536]    f32   (columns: q | k | v, each 512 wide)
  w_out [512, 512]     f32
  b_out [512]          f32

Sharding over 8 cores: core c handles batch (c // 2) and head-group
(c % 2) -> 4 heads of 64 dims each (feature slice of 256 per section).
Each core computes a partial output projection (its 4 heads' contribution
to out = attn @ w_out); the host sums the two partials per batch and adds
the bias.

Per-core dataflow (bf16 matmuls, all intermediates in SBUF):
  xT [512, 2048] -> qT,kT [256+256, 2048] (transposed layout, bf16)
                 -> v natural [2048, 4*65] (65th col per head = ones)
  attention, processed per (query-half of 1024) x (head pair):
    S^T[j-block 128, i 512x2] = kT.T @ qT  two heads packed side by side
                                           (64x128 PE row tiles, concurrent)
    P^T = exp(S^T * 0.125)                 softmax exp split between
                                           ScalarE (exact LUT) and VectorE
                                           (Schraudolph int16 bit-trick,
                                           ~1.8% rms, on a minority of
                                           chunks) to balance engine load
    outT[f 65, i 512] += v'.T @ P^T        (v' stationary; j-major PSUM
                                            accumulation; row 64 = denom l)
    attnT[f, i] = outT[0:64] * (1/l)       (DVE recip straight from PSUM +
                                            gpsimd broadcast + DVE mul)
  out_partial[i 128, 512] = attnT.T @ w_out -> DRAM, with the final
  projection MMs interleaved into the next query-half's j-loop so the
  PE never sits idle at phase boundaries.
"""

import sys

if "/opt/trn_rl_repo" not in sys.path:
    sys.path.insert(0, "/opt/trn_rl_repo")

import numpy as np

DIM = 512
HEADS = 8
DIM_HEAD = 64
INNER = 512
B, N = 4, 2048
N_CORES = 8
HEADS_PER_CORE = 4
FEAT = HEADS_PER_CORE * DIM_HEAD  # 256 features per core per section
SCALE = DIM_HEAD ** -0.5  # 0.125

N_JB = N // 128  # 16 j-blocks

# Schraudolph approximate-exp constants (bf16 bit-trick):
#   bits16(y) = round(A_S * s + B_S),  y ~= exp(SCALE * s)
# A = SCALE * 2^7 * log2(e); B = 127*2^7 - C with C tuned for min rms
# relative error (~1.8 % rms, 4.2 % max).
A_SCHR = SCALE * 128.0 * 1.4426950408889634
B_SCHR = 16256.0 - 7.5

# j-chunks computed on VectorE via Schraudolph (per 16-j pass); the rest
# go through ScalarE's exact exp LUT. j=0 stays on ScalarE so the PV
# accumulation group's start flag executes first; the pass tail stays on
# ScalarE so the drain isn't gated on the slower DVE chunk latency.
DVE_JS = frozenset((2, 4, 7, 9, 11, 13))

_CACHED = {}


def _build():
    import concourse.mybir as mybir
    import concourse.tile as tile
    from concourse import bacc

    f32 = mybir.dt.float32
    bf16 = mybir.dt.bfloat16
    i16 = mybir.dt.int16
    EXP = mybir.ActivationFunctionType.Exp
    MUL = mybir.AluOpType.mult
    ADD = mybir.AluOpType.add

    nc = bacc.Bacc("TRN2", target_bir_lowering=False, debug=False,
                   num_devices=N_CORES)

    xT = nc.declare_dram_parameter("xT", [DIM, N], f32, isOutput=False)
    wqk = nc.declare_dram_parameter("wqk", [DIM, 2 * FEAT], f32, isOutput=False)
    wv = nc.declare_dram_parameter("wv", [DIM, FEAT], f32, isOutput=False)
    w_out = nc.declare_dram_parameter("w_out", [FEAT, DIM], f32, isOutput=False)
    out = nc.declare_dram_parameter("out", [N, DIM], f32, isOutput=True)

    with tile.TileContext(nc) as tc:
        with (
            tc.tile_pool(name="xb", bufs=1) as xb_pool,
            tc.tile_pool(name="raw", bufs=6) as raw_pool,
            tc.tile_pool(name="wraw", bufs=2) as wraw_pool,
            tc.tile_pool(name="wq", bufs=1) as w_pool,
            tc.tile_pool(name="qkt", bufs=1) as qkt_pool,
            tc.tile_pool(name="vs", bufs=1) as v_pool,
            tc.tile_pool(name="pt", bufs=8) as pt_pool,
            tc.tile_pool(name="attnT", bufs=1) as attnT_pool,
            tc.tile_pool(name="scl", bufs=8) as scl_pool,
            tc.tile_pool(name="dout", bufs=3) as dout_pool,
            tc.tile_pool(name="mm", bufs=3, space="PSUM") as mm_psum,
            tc.tile_pool(name="pv", bufs=2, space="PSUM") as pv_psum,
        ):
            # preload the exp table-set while DMAs run so the first real
            # softmax chunk doesn't pay the ~2.7us ACT_TABLE_LOAD stall
            warm = w_pool.tile([1, 16], f32, tag="warm", name="warm")
            nc.vector.memset(warm[:], 0.0)
            nc.scalar.activation(warm[:], warm[:], EXP)

            # ---- weights: load + convert to bf16 ----
            wqk_t = []
            for k in range(4):
                raw = wraw_pool.tile([128, 2 * FEAT], f32, tag="wraw", name=f"qkraw{k}")
                nc.sync.dma_start(out=raw[:], in_=wqk[k * 128:(k + 1) * 128, :])
                t = w_pool.tile([128, 2 * FEAT], bf16, tag=f"wqk{k}", name=f"wqkb{k}")
                nc.vector.tensor_copy(t[:], raw[:])
                wqk_t.append(t)
            # wv / w_out loads are deferred until after the first x block's
            # DMA so the q/k projection starts as early as possible
            wv_t = []

            def load_wv():
                for k in range(4):
                    raw = wraw_pool.tile([128, FEAT], f32, tag="wraw",
                                         name=f"vraw{k}")
                    nc.sync.dma_start(out=raw[:],
                                      in_=wv[k * 128:(k + 1) * 128, :])
                    t = w_pool.tile([128, FEAT], bf16, tag=f"wv{k}",
                                    name=f"wvb{k}")
                    nc.vector.tensor_copy(t[:], raw[:])
                    wv_t.append(t)

            wo = []

            def load_wo():
                for k in range(2):
                    raw = wraw_pool.tile([128, DIM], f32, tag="wraw",
                                         name=f"woraw{k}")
                    nc.sync.dma_start(out=raw[:],
                                      in_=w_out[k * 128:(k + 1) * 128, :])
                    t = w_pool.tile([128, DIM], bf16, tag=f"wob{k}",
                                    name=f"wob{k}")
                    nc.vector.tensor_copy(t[:], raw[:])
                    wo.append(t)

            # ---- x load + projections, streamed by 512-column blocks ----
            # qkt[m] partition p = feature m*128+p of [q(256) | k(256)], bf16
            qkt = [qkt_pool.tile([128, N], bf16, tag=f"qkt{m}", name=f"qkt{m}")
                   for m in range(4)]
            # v natural layout with a ones column per head: [j 128, 4*65] bf16
            vt = [v_pool.tile([128, 4 * 65], bf16, tag=f"v{j}", name=f"v{j}")
                  for j in range(N_JB)]
            xb = [xb_pool.tile([128, N], bf16, tag=f"xb{k}", name=f"xb{k}")
                  for k in range(4)]

            def proj_qk(m, n, evac_dve=False):
                ncol = slice(n * 512, (n + 1) * 512)
                ps = mm_psum.tile([128, 512], f32, tag="qs", name="psb")
                for k in range(4):
                    nc.tensor.matmul(
                        ps[:],
                        wqk_t[k][:, m * 128:(m + 1) * 128],
                        xb[k][:, ncol],
                        start=(k == 0), stop=(k == 3),
                    )
                # pre-phase projections evacuate on the then-idle ScalarE;
                # the pops interleaved into the attention stream use DVE
                # (ScalarE is the busier engine there)
                if evac_dve:
                    nc.vector.tensor_copy(qkt[m][:, ncol], ps[:])
                else:
                    nc.scalar.copy(qkt[m][:, ncol], ps[:])

            def proj_v(j):
                ps = mm_psum.tile([128, 256], f32, tag="qs", name="psv")
                for k in range(4):
                    nc.tensor.matmul(
                        ps[:],
                        xb[k][:, j * 128:(j + 1) * 128],
                        wv_t[k][:],
                        start=(k == 0), stop=(k == 3),
                    )
                v_view = vt[j].rearrange("p (h f) -> p h f", h=4)
                nc.scalar.copy(
                    v_view[:, :, 0:64], ps.rearrange("p (h f) -> p h f", h=4)
                )
                nc.vector.memset(v_view[:, :, 64:65], 1.0)

            # pair-0 q/k and ALL of v first: everything attention pass 1
            # needs. pair-1's projections are deferred and interleaved into
            # the first attention pass via pending_proj.
            # prefetch ALL x blocks up front (wv right after block 0, wo
            # last) so the projection loop never waits on a DMA mid-phase
            def fetch_x(n):
                ncol = slice(n * 512, (n + 1) * 512)
                for k in range(4):
                    raw = raw_pool.tile([128, 512], f32, tag="raw",
                                        name=f"xraw{n}{k}")
                    nc.sync.dma_start(out=raw[:],
                                      in_=xT[k * 128:(k + 1) * 128, ncol])
                    nc.vector.tensor_copy(xb[k][:, ncol], raw[:])

            fetch_x(0)
            load_wv()
            for n in range(1, 4):
                fetch_x(n)
            load_wo()
            for n in range(4):
                proj_qk(0, n)
                proj_qk(2, n)
                for jj in range(4):
                    proj_v(4 * n + jj)
            pending_proj = [(1, 0), (3, 0), (1, 1), (3, 1),
                            (1, 2), (3, 2), (1, 3), (3, 3)]

            # ---- attention + output projection ----
            # attnT[t] partition p = output feature t*128+p, columns = queries
            attnT = [attnT_pool.tile([128, N], bf16, tag=f"aT{t}", name=f"aT{t}")
                     for t in range(2)]

            # Normalize in two stages. Evac: two base-aligned copies move the
            # accumulator out of PSUM right at the pass boundary (the big
            # attn-rows copy on DVE, whose queue is empty there; the denom
            # row on ScalarE) so the accumulator buffers recycle within ~1
            # chunk. Compute: reciprocal (custom DVE op, base-0 SBUF only),
            # gpsimd broadcast, and the multiply run later from SBUF, spread
            # into steps where the engines have slack.
            def norm_evac(ic_ps):
                lrow = scl_pool.tile([1, 512], f32, tag="lrow", name="lrow")
                nc.scalar.copy(lrow[:], ic_ps[64:65, :])
                a_s = scl_pool.tile([64, 512], f32, tag="as", name="a_s")
                nc.vector.tensor_copy(a_s[:], ic_ps[0:64, :])
                return lrow, a_s

            def norm_compute(pair, hh, lrow, a_s, i0):
                rl = scl_pool.tile([1, 512], f32, tag="rl", name="rl")
                nc.vector.reciprocal_approx_fast(rl[:], lrow[:])
                rlb = scl_pool.tile([64, 512], f32, tag="rlb", name="rlb")
                nc.gpsimd.partition_broadcast(rlb[:], rl[:])
                nc.vector.tensor_tensor(
                    attnT[pair][hh * 64:(hh + 1) * 64, i0:i0 + 512],
                    a_s[:], rlb[:], MUL,
                )

            def out_proj_block(ic):
                # one 128-query block of the final projection
                ps = mm_psum.tile([128, 512], f32, tag="qs", name="psd")
                for t in range(2):
                    nc.tensor.matmul(
                        ps[:],
                        attnT[t][:, ic * 128:(ic + 1) * 128],
                        wo[t][:],
                        start=(t == 0), stop=(t == 1),
                    )
                ot = dout_pool.tile([128, 512], f32, tag="ot", name="ot")
                # alternate the evacuation between the two elementwise
                # engines to keep their loads level
                if ic % 2 == 0:
                    nc.scalar.copy(ot[:], ps[:])
                else:
                    nc.vector.tensor_copy(ot[:], ps[:])
                nc.sync.dma_start(out=out[ic * 128:(ic + 1) * 128, :],
                                  in_=ot[:])

            # One continuous software-pipelined stream over all 8 passes x 16
            # key blocks = 128 chunks. Stages: S(c) | exp(c-1) | PV(c-2, one
            # extra step for DVE chunks). Pass boundaries are just parameter
            # changes - the PE queue never drains between passes. Deferred
            # queues keep the DVE/ACT streams dense:
            #   pending_out  - final-projection blocks, queued once the
            #                  normalize writing their attnT columns emitted
            pending_out = []

            passes = [(half, ic, pair)
                      for half in range(2) for ic in range(2)
                      for pair in range(2)]
            NP = len(passes)
            outp_of = {}
            pv_wait = {}
            pv_left = {p: N_JB for p in range(NP)}
            pv_first = {p: True for p in range(NP)}

            def emit_pv(c):
                qs_c, p_c, pi, j = pv_wait.pop(c)
                _, _, pair = passes[pi]
                outp = outp_of[pi]
                first = pv_first[pi]
                pv_first[pi] = False
                pv_left[pi] -= 1
                for hh in range(2):
                    h = 2 * pair + hh
                    nc.tensor.matmul(
                        outp[hh][0:65, :],
                        vt[j][:, h * 65:(h + 1) * 65],
                        p_c[:, hh * 512:(hh + 1) * 512],
                        start=first,
                        stop=(pv_left[pi] == 0),
                    )

            norm_compute_pending = []

            def queue_norms(pi):
                half, ic, pair = passes[pi]
                i0 = half * 1024 + ic * 512
                for hh in range(2):
                    lrow, a_s = norm_evac(outp_of[pi][hh])

                    def run(hh=hh, lrow=lrow, a_s=a_s, pair=pair, i0=i0,
                            half=half, ic=ic):
                        norm_compute(pair, hh, lrow, a_s, i0)
                        # the final projection for these attnT columns may
                        # only be EMITTED after the normalize writing them
                        # (Tile tracks deps by emission order).
                        if hh == 1 and pair == 1:
                            pending_out.extend(
                                range(half * 8 + ic * 4,
                                      half * 8 + ic * 4 + 4))
                    norm_compute_pending.append(run)

            NC_TOT = NP * N_JB
            for c in range(NC_TOT + 4):
                if c < NC_TOT:
                    pi, j = c // N_JB, c % N_JB
                    half, ic, pair = passes[pi]
                    if j == 0:
                        outp_of[pi] = [
                            pv_psum.tile([65, 512], f32, tag="pv",
                                         name=f"o{pi}{hh}")
                            for hh in range(2)]
                    qt, kt = qkt[pair], qkt[2 + pair]
                    i0 = half * 1024 + ic * 512
                    qs = mm_psum.tile([128, 1024], f32, tag="qs", name="qs")
                    # two heads concurrently in 64x128 PE row-tiles
                    for hh in range(2):
                        nc.tensor.matmul(
                            qs[:, hh * 512:(hh + 1) * 512],
                            kt[hh * 64:(hh + 1) * 64, j * 128:(j + 1) * 128],
                            qt[hh * 64:(hh + 1) * 64, i0:i0 + 512],
                            start=True, stop=True,
                        )
                    pv_wait[c] = [qs, None, pi, j]
                e = c - 1
                if 0 <= e < NC_TOT:
                    qs_e = pv_wait[e][0]
                    p = pt_pool.tile([128, 1024], bf16, tag="pt",
                                     name="ptile")
                    if e % N_JB in DVE_JS:
                        nc.vector.tensor_scalar(
                            p[:].bitcast(i16), qs_e[:],
                            A_SCHR, B_SCHR, MUL, ADD,
                        )
                    else:
                        nc.scalar.activation(p[:], qs_e[:], EXP, scale=SCALE)
                    pv_wait[e][1] = p
                for cc in sorted(pv_wait):
                    if cc <= c - 2 - (1 if cc % N_JB in DVE_JS else 0):
                        emit_pv(cc)
                # a pass's accumulation just finished -> evacuate its
                # accumulators right away (frees the PSUM buffers for the
                # next pass within ~1 chunk); the normalize arithmetic is
                # deferred into later steps.
                if c >= 2 and (c - 2) % N_JB == N_JB - 1:
                    done_pi = (c - 2) // N_JB
                    if pv_left.get(done_pi) == 0:
                        queue_norms(done_pi)
                if c % 2 == 0 and c >= 2:
                    if norm_compute_pending and c % N_JB >= 4:
                        norm_compute_pending.pop(0)()
                    elif pending_proj:
                        proj_qk(*pending_proj.pop(0))
                    elif pending_out and (c < NC_TOT - 6 or
                                          len(pending_out) > 2):
                        # keep two blocks in reserve to fill the PE while
                        # the final normalize chains drain
                        out_proj_block(pending_out.pop(0))
            while norm_compute_pending:
                norm_compute_pending.pop(0)()
            while pending_out:
                out_proj_block(pending_out.pop(0))

    nc.compile()
    return nc


def _get_nc():
    if "nc" not in _CACHED:
        _CACHED["nc"] = _build()
    return _CACHED["nc"]


def kernel(x, w_qkv, w_out, b_out):
    from concourse.bass_utils import run_bass_kernel_spmd

    x = np.asarray(x, dtype=np.float32)
    w_qkv = np.asarray(w_qkv, dtype=np.float32)
    w_out = np.asarray(w_out, dtype=np.float32)
    b_out = np.asarray(b_out, dtype=np.float32)

    in_maps = []
    for c in range(N_CORES):
        bi, hg = c // 2, c % 2
        f0 = hg * FEAT
        wq = w_qkv[:, f0:f0 + FEAT]
        wk = w_qkv[:, INNER + f0:INNER + f0 + FEAT]
        wvs = w_qkv[:, 2 * INNER + f0:2 * INNER + f0 + FEAT]
        in_maps.append({
            "xT": np.ascontiguousarray(x[bi].T),
            "wqk": np.ascontiguousarray(np.concatenate([wq, wk], axis=1)),
            "wv": np.ascontiguousarray(wvs),
            "w_out": np.ascontiguousarray(w_out[f0:f0 + FEAT, :]),
        })

    nc = _get_nc()
    res = run_bass_kernel_spmd(nc, in_maps, list(range(N_CORES)))

    outa = np.empty((B, N, DIM), dtype=np.float32)
    for bi in range(B):
        outa[bi] = (res.results[2 * bi]["out"]
                    + res.results[2 * bi + 1]["out"] + b_out)
    return outa
